# revision 7
# baseline (speedup 1.0000x reference)
"""Segment-reduce (min/max/mean per contiguous span) on 8 Trainium2 cores.

Strategy: pure data parallel — core b handles batch b. The host transposes
each batch to feature-major layout A[p, 4t+c] = x[t, 128c+p] so one span is a
single contiguous [128, 4L] SBUF slice; every span then needs exactly one DVE
reduce per statistic (min / max / sum), with the span boundaries baked into
the per-core program at build time (span_idxs is host data when kernel() is
called). Mean = sum * (1/L) as one [128, 1024] tensor-tensor multiply against
a host-provided reciprocal-length tile.

The per-core programs differ (different span boundaries), so instead of one
SPMD NEFF we build 8 specialized programs and dispatch each to its own
NeuronCore via the same PJRT custom-call primitive run_bass_kernel_spmd uses
under axon (run_bass_via_pjrt's single-core path, with jax.default_device
pinning core b).
"""

import sys
import threading

sys.path.insert(0, "/opt/trn_rl_repo")

import numpy as np

B, T, D, S = 8, 4096, 512, 256
PIECES = 4
PT = T // PIECES  # tokens per input-DMA piece


def _spans(span_starts):
    # Reference segments tokens by searchsorted(starts) — span s covers
    # [starts[s], starts[s+1]-1] (last span runs to T-1).
    starts = span_starts.astype(np.int64)
    ends = np.empty_like(starts)
    ends[:-1] = starts[1:] - 1
    ends[-1] = T - 1
    return starts, ends


def _build_program(starts, ends):
    import concourse.bass as bass
    import concourse.mybir as mybir

    f32 = mybir.dt.float32
    X = mybir.AxisListType.X
    nc = bass.Bass(target_bir_lowering=False)
    A = nc.dram_tensor("A", [128, 4 * T], f32, kind="ExternalInput")
    R = nc.dram_tensor("R", [128, 4 * S], f32, kind="ExternalInput")
    MN = nc.dram_tensor("MN", [128, 4 * S], f32, kind="ExternalOutput")
    MX = nc.dram_tensor("MX", [128, 4 * S], f32, kind="ExternalOutput")
    ME = nc.dram_tensor("ME", [128, 4 * S], f32, kind="ExternalOutput")

    lvl_of = [int(e // PT) for e in ends]
    order = sorted(range(S), key=lambda s: (lvl_of[s], starts[s]))

    with (
        nc.Block() as block,
        nc.semaphore("r_sem") as r_sem,
        nc.semaphore("p0_sem") as p0_sem,
        nc.semaphore("p1_sem") as p1_sem,
        nc.semaphore("p2_sem") as p2_sem,
        nc.semaphore("p3_sem") as p3_sem,
        nc.semaphore("o_sem") as o_sem,
        nc.semaphore("v_sem") as v_sem,
        nc.sbuf_tensor("A_sb", [128, 4 * T], f32) as A_sb,
        nc.sbuf_tensor("R_sb", [128, 4 * S], f32) as R_sb,
        nc.sbuf_tensor("MN_sb", [128, 4 * S], f32) as MN_sb,
        nc.sbuf_tensor("MX_sb", [128, 4 * S], f32) as MX_sb,
        nc.sbuf_tensor("SM_sb", [128, 4 * S], f32) as SM_sb,
        nc.sbuf_tensor("ME_sb", [128, 4 * S], f32) as ME_sb,
    ):
        p_sems = [p0_sem, p1_sem, p2_sem, p3_sem]

        @block.gpsimd
        def _(g):
            g.dma_start(R_sb[:], R[:]).then_inc(r_sem, 16)
            for k in range(PIECES):
                g.dma_start(
                    A_sb[:, 4 * PT * k : 4 * PT * (k + 1)],
                    A[:, 4 * PT * k : 4 * PT * (k + 1)],
                ).then_inc(p_sems[k], 16)
            g.wait_ge(v_sem, 1)
            g.dma_start(MN[:], MN_sb[:]).then_inc(o_sem, 16)
            g.dma_start(MX[:], MX_sb[:]).then_inc(o_sem, 16)
            g.dma_start(ME[:], ME_sb[:]).then_inc(o_sem, 16)
            g.wait_ge(o_sem, 48)

        @block.vector
        def _(v):
            cur = -1
            for s in order:
                while lvl_of[s] > cur:
                    cur += 1
                    v.wait_ge(p_sems[cur], 16)
                a, b = int(starts[s]), int(ends[s])
                seg = A_sb[:, 4 * a : 4 * (b + 1)].rearrange("p (t c) -> p c t", c=4)
                nc.vector.tensor_reduce(
                    MN_sb[:, 4 * s : 4 * s + 4], seg, axis=X, op=mybir.AluOpType.min
                )
                nc.vector.tensor_reduce(
                    MX_sb[:, 4 * s : 4 * s + 4], seg, axis=X, op=mybir.AluOpType.max
                )
                nc.vector.tensor_reduce(
                    SM_sb[:, 4 * s : 4 * s + 4], seg, axis=X, op=mybir.AluOpType.add
                )
            v.wait_ge(r_sem, 16)
            v.drain()
            nc.vector.tensor_mul(ME_sb[:], SM_sb[:], R_sb[:]).then_inc(v_sem, 1)

    return nc


class CoreRunner:
    """jit-once runner for one specialized program on one NeuronCore.

    Mirrors bass2jax.run_bass_via_pjrt's single-core path but keeps the
    jitted callable so repeated executions don't re-lower/re-compile.
    """

    def __init__(self, nc, device, core_id):
        import jax
        import concourse.mybir as mybir
        from concourse.bass2jax import install_neuronx_cc_hook, _bass_exec_p

        install_neuronx_cc_hook()
        self.device = device
        self.core_id = core_id
        self.pid_name = (
            nc.partition_id_tensor.name if nc.partition_id_tensor is not None else None
        )
        self.in_names = []
        self.out_names = []
        out_avals = []
        self.zero_outs = []
        for alloc in nc.m.functions[0].allocations:
            if not isinstance(alloc, mybir.MemoryLocationSet):
                continue
            name = alloc.memorylocations[0].name
            if alloc.kind == "ExternalInput":
                self.in_names.append(name)
            elif alloc.kind == "ExternalOutput":
                self.out_names.append(name)
                shape = tuple(alloc.tensor_shape)
                dt = mybir.dt.np(alloc.dtype)
                out_avals.append(jax.core.ShapedArray(shape, dt))
                self.zero_outs.append(np.zeros(shape, dt))
        all_in = tuple(self.in_names + self.out_names)
        n_params = len(self.in_names)
        out_names = tuple(self.out_names)
        out_avals_t = tuple(out_avals)

        def _body(*args):
            return tuple(
                _bass_exec_p.bind(
                    *args,
                    out_avals=out_avals_t,
                    in_names=all_in,
                    out_names=out_names,
                    lowering_input_output_aliases=(),
                    sim_require_finite=True,
                    sim_require_nnan=True,
                    nc=nc,
                )
            )

        self._jit = jax.jit(
            _body, donate_argnums=tuple(range(n_params, n_params + len(out_names)))
        )

    def start(self, in_map):
        """Dispatch asynchronously; returns jax arrays."""
        import jax

        if self.pid_name is not None:
            in_map = {**in_map, self.pid_name: np.array([[self.core_id]], np.uint32)}
        with jax.default_device(self.device):
            args = [np.asarray(in_map[n]) for n in self.in_names] + [
                z.copy() for z in self.zero_outs
            ]
            return self._jit(*args)

    def finish(self, out_arrs):
        return {n: np.asarray(a) for n, a in zip(self.out_names, out_arrs)}


_RUNNERS = None
_RUNNER_META = None
_LOCK = threading.Lock()


def _get_runners(span_idxs):
    """Build + jit the 8 per-core programs (cached on span structure)."""
    global _RUNNERS, _RUNNER_META
    key = span_idxs.tobytes()
    with _LOCK:
        if _RUNNERS is not None and _RUNNER_META[0] == key:
            return _RUNNERS, _RUNNER_META[1]
        import jax

        devs = jax.devices()[:B]
        spans = [_spans(span_idxs[b, :, 0]) for b in range(B)]
        runners = []
        for b in range(B):
            nc = _build_program(*spans[b])
            runners.append(CoreRunner(nc, devs[b], b))
        _RUNNERS = runners
        _RUNNER_META = (key, spans)
        return runners, spans


def _pack_inputs(input, spans):
    in_maps = []
    for b in range(B):
        starts, ends = spans[b]
        A_b = np.ascontiguousarray(
            input[b].reshape(T, 4, 128).transpose(2, 0, 1).reshape(128, 4 * T)
        )
        lens = (ends - starts + 1).astype(np.float32)
        R_b = np.ascontiguousarray(
            np.broadcast_to(np.repeat(1.0 / lens, 4)[None, :], (128, 4 * S))
        )
        in_maps.append({"A": A_b, "R": R_b})
    return in_maps


def _unpack(res_b):
    def fix(M):
        return M.reshape(128, S, 4).transpose(1, 2, 0).reshape(S, D)

    return np.concatenate(
        [fix(res_b["MN"]), fix(res_b["MX"]), fix(res_b["ME"])], axis=-1
    )


def kernel(input, lengths, span_idxs):
    input = np.asarray(input, dtype=np.float32)
    lengths = np.asarray(lengths, dtype=np.int32)
    span_idxs = np.asarray(span_idxs, dtype=np.int32)

    runners, spans = _get_runners(span_idxs)
    in_maps = _pack_inputs(input, spans)

    # Dispatch all 8 cores (async), then collect.
    pending = [None] * B

    def launch(b):
        pending[b] = runners[b].start(in_maps[b])

    threads = [threading.Thread(target=launch, args=(b,)) for b in range(B)]
    for t in threads:
        t.start()
    for t in threads:
        t.join()

    out = np.zeros((B, S, 3 * D), np.float32)
    for b in range(B):
        out[b] = _unpack(runners[b].finish(pending[b]))

    valid = ~((span_idxs[..., 0] == 0) & (span_idxs[..., 1] == 0)) & (
        np.arange(S)[None, :] < lengths[:, None]
    )
    out[~valid] = 0.0
    return out
